# revision 42
# baseline (speedup 1.0000x reference)
"""Trainium2 Bass kernel for GQA attention (B=2, S=2048, D=2048, H=32, KV=8, HD=64).

Sharding over 8 NeuronCores: batch (2) x 4-way head tensor-parallel.
Core c handles batch c//4 and KV heads {2r, 2r+1} (r = c%4) with their
8 query heads. After attention, 4-core AllGathers (one per head-pair
half) assemble the full attention output (transposed layout) and each
core computes a 512-column shard of the final wo projection.

All matmuls run in bf16 (inputs converted host-side), accumulation fp32.

Changes vs the first working version:
- weights are host-prearranged into [128, ...] panels so each loads with
  a single contiguous DMA; xT streams split across the sync and gpsimd
  queues (per-DMA issue cost starved phase A before).
- QT0 projection is interleaved with the K projection inside the xT
  stream loop (8 psum banks: K in the scores region, QT0 in quad) so
  the PE tracks the DMA stream without idling; V then rotates over the
  4 freed scores banks; drains free psum via one big DVE copy so ropes
  never gate the PE.
- attention runs 1-chunk "minis" on two alternating 2-bank score tiles:
  a chunk's scores only wait exp(t-2), so the exp of t-1 overlaps and
  the scores->exp->PV chain is no longer serialized on one psum region.
- causal masks are seeded as a -1e5 triangle bias matmul (trib^T via an
  identity moving operand) that the scores accumulate onto; exp
  underflows the above-diagonal region to exactly 0, removing all mask
  TTs and the eAB->mask->PV dependency.
- V carries 64 all-ones columns so the PV matmul replicates the softmax
  denominator across psum partitions 64:128; normalization is one
  aligned reciprocal + two psum-reading TTs (no partition broadcast).
- gathered-attention chunks load with one rearranged DMA per (pair,
  half), kept off the queues that carry AllGather triggers; osb/out
  drains run on the scalar engine.
"""

import numpy as np
import ml_dtypes

import concourse.bass as bass
import concourse.mybir as mybir
import concourse.tile as tile
from concourse import bacc
from concourse.bass_utils import run_bass_kernel_spmd

B, S, D = 2, 2048, 2048
H, KV, HD = 32, 8, 64
NREP = H // KV
P = 128
NCORES = 8
GRP = 4                  # cores per batch group
QCOLS = 8 * HD           # 512 query cols per core
KCOLS = 2 * HD           # 128 k/v cols per core
OCOLS = D // GRP         # 512 output cols per core
DCH = D // P             # 16 contraction chunks
NJQ = S // 512           # 4 q windows
NPAIR = 4                # head pairs per core (one per QT tile)

bf16 = mybir.dt.bfloat16
f32 = mybir.dt.float32
MULT = mybir.AluOpType.mult
ADD = mybir.AluOpType.add
EXP = mybir.ActivationFunctionType.Exp

_BF = ml_dtypes.bfloat16

_DEBUG_DUMP = False


def build_graph():
    nc = bacc.Bacc("TRN2", target_bir_lowering=False, debug=False, num_devices=NCORES)

    xT = nc.dram_tensor("xT", [D, S], bf16, kind="ExternalInput")
    wq = nc.dram_tensor("wq", [P, NPAIR * DCH * P], bf16, kind="ExternalInput")
    wk = nc.dram_tensor("wk", [P, DCH * KCOLS], bf16, kind="ExternalInput")
    wv = nc.dram_tensor("wv", [P, DCH * KCOLS], bf16, kind="ExternalInput")
    wo = nc.dram_tensor("wo", [P, DCH * OCOLS], bf16, kind="ExternalInput")
    cos4 = nc.dram_tensor("cos4", [P, S], bf16, kind="ExternalInput")
    sin4 = nc.dram_tensor("sin4", [P, S], bf16, kind="ExternalInput")
    trib = nc.dram_tensor("trib", [P, P], bf16, kind="ExternalInput")
    ident = nc.dram_tensor("ident", [P, P], bf16, kind="ExternalInput")
    out = nc.dram_tensor("out", [S, OCOLS], f32, kind="ExternalOutput")
    dbg = None
    if _DEBUG_DUMP:
        dbg = {
            "kt_d": nc.dram_tensor("kt_d", [P, S], bf16, kind="ExternalOutput"),
            "qt0_d": nc.dram_tensor("qt0_d", [P, S], bf16, kind="ExternalOutput"),
            "qt1_d": nc.dram_tensor("qt1_d", [P, S], bf16, kind="ExternalOutput"),
            "v_d": nc.dram_tensor("v_d", [P, DCH * 2 * 65], bf16, kind="ExternalOutput"),
            "att0_d": nc.dram_tensor("att0_d", [P, S], bf16, kind="ExternalOutput"),
            "att3_d": nc.dram_tensor("att3_d", [P, S], bf16, kind="ExternalOutput"),
        }

    with tile.TileContext(nc) as tc:
        _build_body(tc, nc, xT, wq, wk, wv, wo, cos4, sin4, trib, ident, out, dbg)
    nc.compile()
    return nc


def _build_body(tc, nc, xT, wq, wk, wv, wo, cos4, sin4, trib, ident, out,
                dbg=None):
    from contextlib import ExitStack

    with ExitStack() as ctx:
        const = ctx.enter_context(tc.tile_pool(name="const", bufs=1))
        dram = ctx.enter_context(tc.tile_pool(name="dram", bufs=1, space="DRAM"))

        wk_sb = const.tile([P, DCH, KCOLS], bf16)
        wq_sb = [const.tile([P, DCH, P], bf16, name=f"wq{t}") for t in range(NPAIR)]
        wv_sb = const.tile([P, DCH, KCOLS], bf16)
        cos_sb = const.tile([P, NJQ, 512], bf16)
        sin_sb = const.tile([P, NJQ, 512], bf16)
        trib_sb = const.tile([P, P], bf16)
        ident_sb = const.tile([P, P], bf16)

        # PSUM: quad = 4 general banks (projection / PV accumulators),
        # scores region = the other 4.
        pps = ctx.enter_context(tc.tile_pool(name="pps", bufs=1, space="PSUM"))
        scp = ctx.enter_context(tc.tile_pool(name="scps", bufs=1, space="PSUM"))
        ex = ctx.enter_context(tc.tile_pool(name="ex", bufs=1))
        nrm = ctx.enter_context(tc.tile_pool(name="nrm", bufs=1))
        # the 4 general banks as TWO tiles: projection fillers (qA) and PV
        # accumulators (qB) — separate tiles so Tile's tile-granular
        # dependency tracking never serializes one against the other
        qA = pps.tile([P, 2, 512], f32, name="quadA")
        qB = pps.tile([P, 2, 512], f32, name="quadB")

        proj_ctx = ExitStack()
        proj = proj_ctx.enter_context(tc.tile_pool(name="proj", bufs=1))
        xt = [proj.tile([P, S], bf16, name=f"x{c}", tag=f"x{c}") for c in range(DCH)]

        # input DMAs spread across four queues, ordered by first use
        nc.gpsimd.dma_start(wk_sb[:], wk.ap().rearrange("p (c k) -> p c k", c=DCH))
        nc.gpsimd.dma_start(
            wq_sb[0][:],
            wq.ap()[:, 0 : DCH * P].rearrange("p (c k) -> p c k", c=DCH),
        )
        # first chunk split in half so the first K/Q matmuls start sooner
        nc.sync.dma_start(xt[0][:, 0:1024], xT.ap()[0:P, 0:1024])
        nc.sync.dma_start(xt[0][:, 1024:2048], xT.ap()[0:P, 1024:2048])
        for c in range(1, 6):
            nc.sync.dma_start(xt[c][:], xT.ap()[c * P : (c + 1) * P, :])
        for c in range(6, 14):
            nc.scalar.dma_start(xt[c][:], xT.ap()[c * P : (c + 1) * P, :])
        nc.gpsimd.dma_start(cos_sb[:], cos4.ap().rearrange("p (j q) -> p j q", j=NJQ))
        nc.gpsimd.dma_start(sin_sb[:], sin4.ap().rearrange("p (j q) -> p j q", j=NJQ))
        for c in range(14, 16):
            nc.gpsimd.dma_start(xt[c][:], xT.ap()[c * P : (c + 1) * P, :])
        nc.gpsimd.dma_start(wv_sb[:], wv.ap().rearrange("p (c k) -> p c k", c=DCH))
        for t in range(1, NPAIR):
            nc.gpsimd.dma_start(
                wq_sb[t][:],
                wq.ap()[:, t * DCH * P : (t + 1) * DCH * P].rearrange(
                    "p (c k) -> p c k", c=DCH
                ),
            )
        nc.gpsimd.dma_start(trib_sb[:], trib.ap())
        nc.gpsimd.dma_start(ident_sb[:], ident.ap())

        # long-lived activation tensors
        QT = [const.tile([P, NJQ, 512], bf16, name=f"qt{t}") for t in range(NPAIR)]
        KT = const.tile([P, NJQ, 512], bf16, name="kt")
        V = const.tile([P, DCH, 2, 128], bf16, name="vsb")
        attT = [const.tile([P, S], bf16, name=f"attT{t}") for t in range(NPAIR)]

        # cols 64:128 are all-ones: the PV matmul then replicates the
        # softmax denominator across psum partitions 64:128 for free
        nc.vector.memset(V[:, :, :, 64:128], 1.0)

        def rope_raw(dst, raw, b0):
            """dst = raw*cos + swap32(raw)*sin over trig banks [b0:b0+2].
            6 bf16 DVE ops reading the SBUF drain (not psum). (GpSimd
            offload was tried and reverted: DVE/GpSimd SBUF port
            contention slowed both ~3x.)"""
            cw = cos_sb[:, b0 : b0 + 2, :]
            sw = sin_sb[:, b0 : b0 + 2, :]
            ra = proj.tile([P, 2, 512], bf16, tag="ra", name="ra", bufs=2)
            rb = proj.tile([P, 2, 512], bf16, tag="rb", name="rb", bufs=2)
            nc.vector.tensor_tensor(out=ra[:], in0=raw[:], in1=cw, op=MULT)
            for ob, ib in ((0, 32), (32, 0), (64, 96), (96, 64)):
                nc.vector.tensor_tensor(
                    out=rb[ob : ob + 32, :, :],
                    in0=raw[ib : ib + 32, :, :],
                    in1=sw[ib : ib + 32, :, :],
                    op=MULT,
                )
            nc.vector.tensor_tensor(out=dst, in0=ra[:], in1=rb[:], op=ADD)

        def drain_rope(ps_groups, dst_t, tag):
            """Full 4-bank drain + rope on DVE: both 2-bank drain copies
            first (frees the psum fast), then the ropes read the copies.
            ps_groups: two [P, 2, 512] psum APs/tiles."""
            raws = []
            for g in range(2):
                raw = proj.tile([P, 2, 512], bf16, tag=f"raw{tag}{g}", bufs=1)
                nc.vector.tensor_copy(out=raw[:], in_=ps_groups[g][:])
                raws.append(raw)
            for g in range(2):
                rope_raw(dst_t[:, 2 * g : 2 * g + 2, :], raws[g], 2 * g)

        # ---- phase A: K + QT0 track the xT stream, then V ------------
        kps = [scp.tile([P, 2, 512], f32, tag=f"sa{g}", name=f"kps{g}", bufs=1)
               for g in range(2)]
        for c in range(DCH):
            for js in range(4):
                nc.tensor.matmul(
                    kps[js // 2][:, js % 2, :], wk_sb[:, c, :],
                    xt[c][:, js * 512 : (js + 1) * 512],
                    start=(c == 0), stop=(c == DCH - 1),
                )
            for js in range(4):
                qt_ps = qA if js < 2 else qB
                nc.tensor.matmul(
                    qt_ps[:, js % 2, :], wq_sb[0][:, c, :],
                    xt[c][:, js * 512 : (js + 1) * 512],
                    start=(c == 0), stop=(c == DCH - 1),
                )
        # drains free the psum via one big DVE copy per 2-bank group;
        # ropes then run from SBUF while V's matmuls proceed on the PE
        drain_rope(kps, KT, "k")
        # V projection: it-chunks in interleaved pairs so consecutive
        # matmuls alternate psum banks (and tiles) instead of hammering
        # one accumulation chain back-to-back
        for itp in range(DCH // 2):
            it0, it1 = 2 * itp, 2 * itp + 1
            ba, bb = itp % 2, 2 + itp % 2  # bank in kps[0] / kps[1]
            for c in range(DCH):
                nc.tensor.matmul(
                    kps[0][:, ba % 2, 0:128], xt[c][:, it0 * P : (it0 + 1) * P],
                    wv_sb[:, c, :],
                    start=(c == 0), stop=(c == DCH - 1),
                )
                nc.tensor.matmul(
                    kps[1][:, bb % 2, 0:128], xt[c][:, it1 * P : (it1 + 1) * P],
                    wv_sb[:, c, :],
                    start=(c == 0), stop=(c == DCH - 1),
                )
            for i, (it, kb) in enumerate(((it0, kps[0][:, ba % 2, :]),
                                          (it1, kps[1][:, bb % 2, :]))):
                nc.scalar.copy(V[:, it, 0, 0:64], kb[0:128, 0:64])
                nc.scalar.copy(V[:, it, 1, 0:64], kb[0:128, 64:128])
            if itp == 0:
                drain_rope([qA, qB], QT[0], "q")

        def emit_qt(ot):
            """Generator emitting QT[ot] projection in small PE batches.
            One DVE copy frees the psum; rope reads the copy."""
            for jp in range(2):
                jss = (2 * jp, 2 * jp + 1)
                for c in range(DCH):
                    for i, js in enumerate(jss):
                        nc.tensor.matmul(
                            qA[:, i, :], wq_sb[ot][:, c, :],
                            xt[c][:, js * 512 : (js + 1) * 512],
                            start=(c == 0), stop=(c == DCH - 1),
                        )
                    if c % 2 == 1:
                        yield None
                raw = proj.tile([P, 2, 512], bf16, tag="rawf", bufs=2)
                nc.vector.tensor_copy(out=raw[:], in_=qA[:])
                yield None
                rope_raw(QT[ot][:, 2 * jp : 2 * jp + 2, :], raw, 2 * jp)
                yield None

        # ---- phase B: attention, QT[t+1] interleaved -----------------
        att_loc = [[dram.tile([P, 1024], bf16, name=f"aloc{t}_{h}") for h in range(2)]
                   for t in range(NPAIR)]
        att_all = [[dram.tile([GRP * P, 1024], bf16, name=f"aall{t}_{h}") for h in range(2)]
                   for t in range(NPAIR)]
        wos = None
        wo_sb = None
        cht = [[None] * 2 for _ in range(NPAIR)]   # [P, GRP, 1024] per (t, h)


        def emit_norm(pair, jq):
            qw = slice(jq * 512, (jq + 1) * 512)
            # psum partitions 64:128 of the PV banks hold the denominator
            # (replicated by the ones block of V); one aligned reciprocal
            # + two psum-reading TTs finish the softmax
            den = nrm.tile([64, 2, 512], f32, tag="den", name="den", bufs=2)
            nc.vector.tensor_copy(out=den[:], in_=qB[64:128, :, :])
            rec = nrm.tile([64, 2, 512], f32, tag="rec", name="rec", bufs=2)
            nc.vector.reciprocal_approx_fast(out=rec[:], in_=den[:])
            nc.vector.tensor_tensor(
                out=attT[pair][0:64, qw], in0=qB[0:64, 0, :],
                in1=rec[:, 0, :], op=MULT,
            )
            nc.vector.tensor_tensor(
                out=attT[pair][64:128, qw], in0=qB[0:64, 1, :],
                in1=rec[:, 1, :], op=MULT,
            )
            if jq % 2 == 1:  # half complete -> ship + gather
                h = jq // 2
                hw_ = slice(h * 1024, (h + 1) * 1024)
                nc.sync.dma_start(att_loc[pair][h][:], attT[pair][:, hw_])
                nc.gpsimd.collective_compute(
                    "AllGather",
                    mybir.AluOpType.bypass,
                    replica_groups=[[0, 1, 2, 3], [4, 5, 6, 7]],
                    ins=[att_loc[pair][h][:].opt()],
                    outs=[att_all[pair][h][:].opt()],
                )

        def emit_pv(ent):
            eAB_p, ik, nch, pair, jq, Wc = ent
            nc.tensor.matmul(
                qB[:, 0, Wc:512], V[:, ik, 0, :], eAB_p[:, 0, :],
                start=(ik == 0), stop=(ik == nch - 1),
            )
            nc.tensor.matmul(
                qB[:, 1, Wc:512], V[:, ik, 1, :], eAB_p[:, 1, :],
                start=(ik == 0), stop=(ik == nch - 1),
            )
            if ik + 1 >= nch:  # last chunk of this (pair, jq)
                emit_norm(pair, jq)

        def _cht_load(t, h):
            nc.gpsimd.dma_start(
                cht[t][h][:],
                att_all[t][h][:].rearrange("(r p) q -> p r q", p=P),
            )

        def emit_wo_loads():
            # wo weights + gathered-chunk preloads, spread across pair
            # 3's minis. Pair 3's own chunk loads ride the sync queue so
            # a still-flying AG never blocks the gpsimd stream that
            # carries the next AllGather triggers.
            nc.gpsimd.dma_start(
                wo_sb[:], wo.ap().rearrange("p (c k) -> p c k", c=DCH)
            )
            yield None
            for t in range(3):
                _cht_load(t, 0)
                yield None
            for t in range(2):
                _cht_load(t, 1)
                yield None
            for _ in range(8):
                yield None
            _cht_load(2, 1)
            yield None
            for _ in range(6):
                yield None
            nc.sync.dma_start(
                cht[3][0][:],
                att_all[3][0][:].rearrange("(r p) q -> p r q", p=P),
            )
            yield None

        pend = []  # global software pipeline: scores/exp run 1 ahead of PV
        filler = None
        for pair in range(NPAIR):
            filler = emit_qt(pair + 1) if pair + 1 < NPAIR else emit_wo_loads()
            for jq in range(NJQ):
                nch = 4 * jq + 4
                for ik in range(nch):
                    d = ik - 4 * jq
                    diag = d >= 0
                    Wc = 128 * d if diag else 0  # per-chunk causal window
                    # two alternating 2-bank score tiles: this chunk's
                    # scores only wait exp(t-2), so exp(t-1) overlaps
                    sAB = scp.tile([P, 2, 512], f32, tag=f"sa{ik % 2}",
                                   name="sAB", bufs=1)
                    if diag:
                        # seed the causal triangle with -1e5 (trib^T via
                        # identity); exp underflows it to exactly 0
                        for s_ in (0, 1):
                            nc.tensor.matmul(
                                sAB[:, s_, Wc : Wc + 128], trib_sb[:],
                                ident_sb[:], start=True, stop=False,
                            )
                    kap = KT[:, ik // 4, (ik % 4) * 128 : (ik % 4 + 1) * 128]
                    qap = QT[pair][:, jq, Wc:512]
                    nc.tensor.matmul(
                        sAB[:, 0, Wc:512], kap[0:64, :], qap[0:64, :],
                        start=not diag, stop=True,
                    )
                    nc.tensor.matmul(
                        sAB[:, 1, Wc:512], kap[64:128, :], qap[64:128, :],
                        start=not diag, stop=True,
                    )
                    eAB = ex.tile([P, 2, 512 - Wc], bf16, tag="eAB",
                                  name="eAB", bufs=5)
                    nc.scalar.activation(eAB[:], sAB[:, :, Wc:512], EXP,
                                         scale=0.125)
                    pend.append((eAB, ik, nch, pair, jq, Wc))
                    if len(pend) > 2:
                        emit_pv(pend.pop(0))
                    if filler is not None:
                        # PE filler; emitted last so its DVE ropes queue
                        # behind the norm copies that gate PV
                        if next(filler, StopIteration) is StopIteration:
                            filler = None
            if filler is not None:  # drain leftover projection work
                for _ in filler:
                    pass
                filler = None
            if pair == 2:
                # drain the PV pipeline so pair 2's last AllGather is
                # emitted before the chunk preloads that read it
                while pend:
                    emit_pv(pend.pop(0))
                # xT no longer needed; free it so the wo tiles can take
                # its address range
                proj_ctx.close()
                wos = ctx.enter_context(tc.tile_pool(name="wos", bufs=1))
                wo_sb = wos.tile([P, DCH, OCOLS], bf16)
                for t in range(NPAIR):
                    for h in range(2):
                        cht[t][h] = wos.tile(
                            [P, GRP, 1024], bf16, name=f"cht{t}_{h}",
                            tag=f"cht{t}_{h}",
                        )
        while pend:
            emit_pv(pend.pop(0))
        # gpsimd is free of critical work now (all AG triggers emitted)
        nc.gpsimd.dma_start(
            cht[3][1][:], att_all[3][1][:].rearrange("(r p) q -> p r q", p=P)
        )

        # ---- phase C: wo projection (lo/hi halves overlap last AGs) --
        # m-rows in interleaved pairs (consecutive matmuls alternate psum
        # banks); 4 independent psum tiles hold 4 m-pairs at once. The
        # contraction is split: t=0..2 chunks for ALL 4 pairs first (96
        # matmuls of runway that need no pair-3 data), then the t=3
        # tails + drains — so a still-flying pair-3 AllGather is hidden
        # behind the runway instead of stalling the whole phase.
        wops = [scp.tile([P, 2, 512], f32, tag=f"sa{g}", name=f"wop{g}", bufs=1)
                for g in range(2)] + [qA, qB]
        for h in range(2):
            for mp in range(4):     # partials: t = 0..2
                for c2 in range(12):
                    t, rr = c2 // 4, c2 % 4
                    for i in range(2):
                        mm = 2 * mp + i
                        nc.tensor.matmul(
                            wops[mp][:, i, :],
                            cht[t][h][:, rr, mm * P : (mm + 1) * P],
                            wo_sb[:, c2, :], start=(c2 == 0), stop=False,
                        )
            for mp in range(4):     # tails: t = 3, then drain
                for c2 in range(12, 16):
                    rr = c2 % 4
                    for i in range(2):
                        mm = 2 * mp + i
                        nc.tensor.matmul(
                            wops[mp][:, i, :],
                            cht[3][h][:, rr, mm * P : (mm + 1) * P],
                            wo_sb[:, c2, :], start=False, stop=(c2 == 15),
                        )
                for i in range(2):
                    m = h * 8 + 2 * mp + i
                    osb = wos.tile([P, OCOLS], f32, tag="osb", name="osb", bufs=4)
                    nc.vector.tensor_copy(out=osb[:], in_=wops[mp][:, i, :])
                    nc.sync.dma_start(
                        out.ap()[m * P : (m + 1) * P, :], osb[:]
                    )

        if dbg is not None:
            nc.sync.dma_start(dbg["kt_d"].ap(), KT[:].rearrange("p j q -> p (j q)"))
            nc.sync.dma_start(dbg["qt0_d"].ap(), QT[0][:].rearrange("p j q -> p (j q)"))
            nc.sync.dma_start(dbg["qt1_d"].ap(), QT[1][:].rearrange("p j q -> p (j q)"))
            nc.sync.dma_start(dbg["v_d"].ap(), V[:].rearrange("p c t v -> p (c t v)"))
            nc.sync.dma_start(dbg["att0_d"].ap(), attT[0][:])
            nc.sync.dma_start(dbg["att3_d"].ap(), attT[3][:])


# ---------------------------------------------------------------------------
# host side
# ---------------------------------------------------------------------------

_PERM64 = np.concatenate([np.arange(0, 64, 2), np.arange(1, 64, 2)])


def _qcols(r):
    cols = []
    for t in range(NREP):
        for half in range(2):
            h = (2 * r + half) * NREP + t
            cols.extend(64 * h + _PERM64)
    return np.array(cols)


def _kcols(r):
    cols = []
    for half in range(2):
        g = 2 * r + half
        cols.extend(64 * g + _PERM64)
    return np.array(cols)


def _worows():
    rows = []
    for t in range(NREP):
        for rr in range(GRP):
            for half in range(2):
                h = (2 * rr + half) * NREP + t
                rows.extend(64 * h + np.arange(64))
    return np.array(rows)


def _panel(w):
    """[DCH*P, N] -> [P, DCH*N] (chunk-major per partition)."""
    n = w.shape[1]
    return np.ascontiguousarray(
        w.reshape(DCH, P, n).transpose(1, 0, 2).reshape(P, DCH * n)
    )


def make_in_maps(x, wq, wk, wv, wo, freqs_cos, freqs_sin):
    cosT = np.ascontiguousarray(freqs_cos.T).astype(np.float32)  # (32, S)
    sinT = np.ascontiguousarray(freqs_sin.T).astype(np.float32)
    cos4 = np.ascontiguousarray(np.tile(cosT, (4, 1))).astype(_BF)  # (128, S)
    # pre-swapped by 32-blocks: partition p holds the sin coefficient
    # for the rope output at partition swap32(p)
    sin4 = np.ascontiguousarray(
        np.concatenate([sinT, -sinT, sinT, -sinT], axis=0)
    ).astype(_BF)
    trib = (-1e5 * np.triu(np.ones((P, P), dtype=np.float32), 1)).astype(_BF)
    ident = np.eye(P, dtype=np.float32).astype(_BF)

    xT = [np.ascontiguousarray(x[b].T).astype(_BF) for b in range(B)]
    wo_perm = wo[_worows(), :]

    in_maps = []
    for c in range(NCORES):
        b, r = c // GRP, c % GRP
        wq_r = wq[:, _qcols(r)].astype(_BF)      # [D, 512], tile-major cols
        wq_pan = np.concatenate(
            [_panel(wq_r[:, t * P : (t + 1) * P]) for t in range(NPAIR)], axis=1
        )
        in_maps.append(
            {
                "xT": xT[b],
                "wq": np.ascontiguousarray(wq_pan),
                "wk": _panel(wk[:, _kcols(r)].astype(_BF)),
                "wv": _panel(wv[:, 128 * r : 128 * (r + 1)].astype(_BF)),
                "wo": _panel(
                    wo_perm[:, OCOLS * r : OCOLS * (r + 1)].astype(_BF)
                ),
                "cos4": cos4,
                "sin4": sin4,
                "trib": trib,
                "ident": ident,
            }
        )
    return in_maps


_NC_CACHE = None


def _get_nc():
    global _NC_CACHE
    if _NC_CACHE is None:
        _NC_CACHE = build_graph()
    return _NC_CACHE


def kernel(x, wq, wk, wv, wo, freqs_cos, freqs_sin):
    x = np.asarray(x)
    wq = np.asarray(wq)
    wk = np.asarray(wk)
    wv = np.asarray(wv)
    wo = np.asarray(wo)
    freqs_cos = np.asarray(freqs_cos)
    freqs_sin = np.asarray(freqs_sin)

    in_maps = make_in_maps(x, wq, wk, wv, wo, freqs_cos, freqs_sin)
    nc = _get_nc()
    res = run_bass_kernel_spmd(nc, in_maps, core_ids=list(range(NCORES)))
    global _LAST_RES
    _LAST_RES = res

    out = np.empty((B, S, D), dtype=np.float32)
    for c in range(NCORES):
        b, r = c // GRP, c % GRP
        out[b, :, OCOLS * r : OCOLS * (r + 1)] = res.results[c]["out"]
    return out



# revision 43
# speedup vs baseline: 1.2905x; 1.2905x over previous
"""Trainium2 Bass kernel for GQA attention (B=2, S=2048, D=2048, H=32, KV=8, HD=64).

Sharding over 8 NeuronCores: batch (2) x 4-way head tensor-parallel.
Core c handles batch c//4 and KV heads {2r, 2r+1} (r = c%4) with their
8 query heads. After attention, 4-core AllGathers (one per head-pair
half) assemble the full attention output (transposed layout) and each
core computes a 512-column shard of the final wo projection.

All matmuls run in bf16 (inputs converted host-side), accumulation fp32.

Changes vs the first working version:
- weights are host-prearranged into [128, ...] panels so each loads with
  a single contiguous DMA; xT streams split across the sync and gpsimd
  queues (per-DMA issue cost starved phase A before).
- QT0 projection is interleaved with the K projection inside the xT
  stream loop (8 psum banks: K in the scores region, QT0 in quad) so
  the PE tracks the DMA stream without idling; V then rotates over the
  4 freed scores banks; drains free psum via one big DVE copy so ropes
  never gate the PE.
- attention runs 1-chunk "minis" on two alternating 2-bank score tiles:
  a chunk's scores only wait exp(t-2), so the exp of t-1 overlaps and
  the scores->exp->PV chain is no longer serialized on one psum region.
- causal masks are seeded as a -1e5 triangle bias matmul (trib^T via an
  identity moving operand) that the scores accumulate onto; exp
  underflows the above-diagonal region to exactly 0, removing all mask
  TTs and the eAB->mask->PV dependency.
- V carries 64 all-ones columns so the PV matmul replicates the softmax
  denominator across psum partitions 64:128; normalization is one
  aligned reciprocal + two psum-reading TTs (no partition broadcast).
- gathered-attention chunks load with one rearranged DMA per (pair,
  half), kept off the queues that carry AllGather triggers; osb/out
  drains run on the scalar engine.
"""

import numpy as np
import ml_dtypes

import concourse.bass as bass
import concourse.mybir as mybir
import concourse.tile as tile
from concourse import bacc
from concourse.bass_utils import run_bass_kernel_spmd

B, S, D = 2, 2048, 2048
H, KV, HD = 32, 8, 64
NREP = H // KV
P = 128
NCORES = 8
GRP = 4                  # cores per batch group
QCOLS = 8 * HD           # 512 query cols per core
KCOLS = 2 * HD           # 128 k/v cols per core
OCOLS = D // GRP         # 512 output cols per core
DCH = D // P             # 16 contraction chunks
NJQ = S // 512           # 4 q windows
NPAIR = 4                # head pairs per core (one per QT tile)

bf16 = mybir.dt.bfloat16
f32 = mybir.dt.float32
MULT = mybir.AluOpType.mult
ADD = mybir.AluOpType.add
EXP = mybir.ActivationFunctionType.Exp

_BF = ml_dtypes.bfloat16

_DEBUG_DUMP = False


def build_graph():
    nc = bacc.Bacc("TRN2", target_bir_lowering=False, debug=False, num_devices=NCORES)

    xT = nc.dram_tensor("xT", [D, S], bf16, kind="ExternalInput")
    wq = nc.dram_tensor("wq", [P, NPAIR * DCH * P], bf16, kind="ExternalInput")
    wk = nc.dram_tensor("wk", [P, DCH * KCOLS], bf16, kind="ExternalInput")
    wv = nc.dram_tensor("wv", [P, DCH * KCOLS], bf16, kind="ExternalInput")
    wo = nc.dram_tensor("wo", [P, DCH * OCOLS], bf16, kind="ExternalInput")
    cos4 = nc.dram_tensor("cos4", [P, S], bf16, kind="ExternalInput")
    sin4 = nc.dram_tensor("sin4", [P, S], bf16, kind="ExternalInput")
    trib = nc.dram_tensor("trib", [P, P], bf16, kind="ExternalInput")
    ident = nc.dram_tensor("ident", [P, P], bf16, kind="ExternalInput")
    out = nc.dram_tensor("out", [S, OCOLS], f32, kind="ExternalOutput")
    dbg = None
    if _DEBUG_DUMP:
        dbg = {
            "kt_d": nc.dram_tensor("kt_d", [P, S], bf16, kind="ExternalOutput"),
            "qt0_d": nc.dram_tensor("qt0_d", [P, S], bf16, kind="ExternalOutput"),
            "qt1_d": nc.dram_tensor("qt1_d", [P, S], bf16, kind="ExternalOutput"),
            "v_d": nc.dram_tensor("v_d", [P, DCH * 2 * 65], bf16, kind="ExternalOutput"),
            "att0_d": nc.dram_tensor("att0_d", [P, S], bf16, kind="ExternalOutput"),
            "att3_d": nc.dram_tensor("att3_d", [P, S], bf16, kind="ExternalOutput"),
        }

    with tile.TileContext(nc) as tc:
        _build_body(tc, nc, xT, wq, wk, wv, wo, cos4, sin4, trib, ident, out, dbg)
    nc.compile()
    return nc


def _build_body(tc, nc, xT, wq, wk, wv, wo, cos4, sin4, trib, ident, out,
                dbg=None):
    from contextlib import ExitStack

    with ExitStack() as ctx:
        const = ctx.enter_context(tc.tile_pool(name="const", bufs=1))
        dram = ctx.enter_context(tc.tile_pool(name="dram", bufs=1, space="DRAM"))

        wk_sb = const.tile([P, DCH, KCOLS], bf16)
        wq_sb = [const.tile([P, DCH, P], bf16, name=f"wq{t}") for t in range(NPAIR)]
        wv_sb = const.tile([P, DCH, KCOLS], bf16)
        cos_sb = const.tile([P, NJQ, 512], bf16)
        sin_sb = const.tile([P, NJQ, 512], bf16)
        trib_sb = const.tile([P, P], bf16)
        ident_sb = const.tile([P, P], bf16)

        # PSUM: quad = 4 general banks (projection / PV accumulators),
        # scores region = the other 4.
        pps = ctx.enter_context(tc.tile_pool(name="pps", bufs=1, space="PSUM"))
        scp = ctx.enter_context(tc.tile_pool(name="scps", bufs=1, space="PSUM"))
        ex = ctx.enter_context(tc.tile_pool(name="ex", bufs=1))
        nrm = ctx.enter_context(tc.tile_pool(name="nrm", bufs=1))
        # the 4 general banks as TWO tiles: projection fillers (qA) and PV
        # accumulators (qB) — separate tiles so Tile's tile-granular
        # dependency tracking never serializes one against the other
        qA = pps.tile([P, 2, 512], f32, name="quadA")
        qB = pps.tile([P, 2, 512], f32, name="quadB")

        proj_ctx = ExitStack()
        proj = proj_ctx.enter_context(tc.tile_pool(name="proj", bufs=1))
        xt = [proj.tile([P, S], bf16, name=f"x{c}", tag=f"x{c}") for c in range(DCH)]

        # input DMAs spread across four queues, ordered by first use
        nc.gpsimd.dma_start(wk_sb[:], wk.ap().rearrange("p (c k) -> p c k", c=DCH))
        nc.gpsimd.dma_start(
            wq_sb[0][:],
            wq.ap()[:, 0 : DCH * P].rearrange("p (c k) -> p c k", c=DCH),
        )
        # first chunk split in half so the first K/Q matmuls start sooner
        nc.sync.dma_start(xt[0][:, 0:1024], xT.ap()[0:P, 0:1024])
        nc.sync.dma_start(xt[0][:, 1024:2048], xT.ap()[0:P, 1024:2048])
        for c in range(1, 10):
            nc.sync.dma_start(xt[c][:], xT.ap()[c * P : (c + 1) * P, :])
        nc.gpsimd.dma_start(cos_sb[:], cos4.ap().rearrange("p (j q) -> p j q", j=NJQ))
        nc.gpsimd.dma_start(sin_sb[:], sin4.ap().rearrange("p (j q) -> p j q", j=NJQ))
        for c in range(10, 16):
            nc.gpsimd.dma_start(xt[c][:], xT.ap()[c * P : (c + 1) * P, :])
        nc.gpsimd.dma_start(wv_sb[:], wv.ap().rearrange("p (c k) -> p c k", c=DCH))
        for t in range(1, NPAIR):
            nc.gpsimd.dma_start(
                wq_sb[t][:],
                wq.ap()[:, t * DCH * P : (t + 1) * DCH * P].rearrange(
                    "p (c k) -> p c k", c=DCH
                ),
            )
        nc.gpsimd.dma_start(trib_sb[:], trib.ap())
        nc.gpsimd.dma_start(ident_sb[:], ident.ap())

        # long-lived activation tensors
        QT = [const.tile([P, NJQ, 512], bf16, name=f"qt{t}") for t in range(NPAIR)]
        KT = const.tile([P, NJQ, 512], bf16, name="kt")
        V = const.tile([P, DCH, 2, 128], bf16, name="vsb")
        attT = [const.tile([P, S], bf16, name=f"attT{t}") for t in range(NPAIR)]

        # cols 64:128 are all-ones: the PV matmul then replicates the
        # softmax denominator across psum partitions 64:128 for free
        nc.vector.memset(V[:, :, :, 64:128], 1.0)

        def rope_raw(dst, raw, b0):
            """dst = raw*cos + swap32(raw)*sin over trig banks [b0:b0+2].
            6 bf16 DVE ops reading the SBUF drain (not psum). (GpSimd
            offload was tried and reverted: DVE/GpSimd SBUF port
            contention slowed both ~3x.)"""
            cw = cos_sb[:, b0 : b0 + 2, :]
            sw = sin_sb[:, b0 : b0 + 2, :]
            ra = proj.tile([P, 2, 512], bf16, tag="ra", name="ra", bufs=2)
            rb = proj.tile([P, 2, 512], bf16, tag="rb", name="rb", bufs=2)
            nc.vector.tensor_tensor(out=ra[:], in0=raw[:], in1=cw, op=MULT)
            for ob, ib in ((0, 32), (32, 0), (64, 96), (96, 64)):
                nc.vector.tensor_tensor(
                    out=rb[ob : ob + 32, :, :],
                    in0=raw[ib : ib + 32, :, :],
                    in1=sw[ib : ib + 32, :, :],
                    op=MULT,
                )
            nc.vector.tensor_tensor(out=dst, in0=ra[:], in1=rb[:], op=ADD)

        def drain_rope(ps_groups, dst_t, tag):
            """Full 4-bank drain + rope on DVE: both 2-bank drain copies
            first (frees the psum fast), then the ropes read the copies.
            ps_groups: two [P, 2, 512] psum APs/tiles."""
            raws = []
            for g in range(2):
                raw = proj.tile([P, 2, 512], bf16, tag=f"raw{tag}{g}", bufs=1)
                nc.vector.tensor_copy(out=raw[:], in_=ps_groups[g][:])
                raws.append(raw)
            for g in range(2):
                rope_raw(dst_t[:, 2 * g : 2 * g + 2, :], raws[g], 2 * g)

        # ---- phase A: K + QT0 track the xT stream, then V ------------
        kps = [scp.tile([P, 2, 512], f32, tag=f"sa{g}", name=f"kps{g}", bufs=1)
               for g in range(2)]
        for c in range(DCH):
            for js in range(4):
                nc.tensor.matmul(
                    kps[js // 2][:, js % 2, :], wk_sb[:, c, :],
                    xt[c][:, js * 512 : (js + 1) * 512],
                    start=(c == 0), stop=(c == DCH - 1),
                )
            for js in range(4):
                qt_ps = qA if js < 2 else qB
                nc.tensor.matmul(
                    qt_ps[:, js % 2, :], wq_sb[0][:, c, :],
                    xt[c][:, js * 512 : (js + 1) * 512],
                    start=(c == 0), stop=(c == DCH - 1),
                )
        # drains free the psum via one big DVE copy per 2-bank group;
        # ropes then run from SBUF while V's matmuls proceed on the PE
        drain_rope(kps, KT, "k")
        # V projection: it-chunks in interleaved pairs so consecutive
        # matmuls alternate psum banks (and tiles) instead of hammering
        # one accumulation chain back-to-back
        for itp in range(DCH // 2):
            it0, it1 = 2 * itp, 2 * itp + 1
            ba, bb = itp % 2, 2 + itp % 2  # bank in kps[0] / kps[1]
            for c in range(DCH):
                nc.tensor.matmul(
                    kps[0][:, ba % 2, 0:128], xt[c][:, it0 * P : (it0 + 1) * P],
                    wv_sb[:, c, :],
                    start=(c == 0), stop=(c == DCH - 1),
                )
                nc.tensor.matmul(
                    kps[1][:, bb % 2, 0:128], xt[c][:, it1 * P : (it1 + 1) * P],
                    wv_sb[:, c, :],
                    start=(c == 0), stop=(c == DCH - 1),
                )
            for i, (it, kb) in enumerate(((it0, kps[0][:, ba % 2, :]),
                                          (it1, kps[1][:, bb % 2, :]))):
                nc.scalar.copy(V[:, it, 0, 0:64], kb[0:128, 0:64])
                nc.scalar.copy(V[:, it, 1, 0:64], kb[0:128, 64:128])
            if itp == 0:
                drain_rope([qA, qB], QT[0], "q")

        def emit_qt(ot):
            """Generator emitting QT[ot] projection in small PE batches.
            One DVE copy frees the psum; rope reads the copy."""
            for jp in range(2):
                jss = (2 * jp, 2 * jp + 1)
                for c in range(DCH):
                    for i, js in enumerate(jss):
                        nc.tensor.matmul(
                            qA[:, i, :], wq_sb[ot][:, c, :],
                            xt[c][:, js * 512 : (js + 1) * 512],
                            start=(c == 0), stop=(c == DCH - 1),
                        )
                    if c % 2 == 1:
                        yield None
                raw = proj.tile([P, 2, 512], bf16, tag="rawf", bufs=2)
                nc.vector.tensor_copy(out=raw[:], in_=qA[:])
                yield None
                rope_raw(QT[ot][:, 2 * jp : 2 * jp + 2, :], raw, 2 * jp)
                yield None

        # ---- phase B: attention, QT[t+1] interleaved -----------------
        att_loc = [[dram.tile([P, 1024], bf16, name=f"aloc{t}_{h}") for h in range(2)]
                   for t in range(NPAIR)]
        att_all = [[dram.tile([GRP * P, 1024], bf16, name=f"aall{t}_{h}") for h in range(2)]
                   for t in range(NPAIR)]
        wos = None
        wo_sb = None
        cht = [[None] * 2 for _ in range(NPAIR)]   # [P, GRP, 1024] per (t, h)


        def emit_norm(pair, jq):
            qw = slice(jq * 512, (jq + 1) * 512)
            # psum partitions 64:128 of the PV banks hold the denominator
            # (replicated by the ones block of V); one aligned reciprocal
            # + two psum-reading TTs finish the softmax
            den = nrm.tile([64, 2, 512], f32, tag="den", name="den", bufs=2)
            nc.vector.tensor_copy(out=den[:], in_=qB[64:128, :, :])
            rec = nrm.tile([64, 2, 512], f32, tag="rec", name="rec", bufs=2)
            nc.vector.reciprocal_approx_fast(out=rec[:], in_=den[:])
            nc.vector.tensor_tensor(
                out=attT[pair][0:64, qw], in0=qB[0:64, 0, :],
                in1=rec[:, 0, :], op=MULT,
            )
            nc.vector.tensor_tensor(
                out=attT[pair][64:128, qw], in0=qB[0:64, 1, :],
                in1=rec[:, 1, :], op=MULT,
            )
            if jq % 2 == 1:  # half complete -> ship + gather
                h = jq // 2
                hw_ = slice(h * 1024, (h + 1) * 1024)
                nc.sync.dma_start(att_loc[pair][h][:], attT[pair][:, hw_])
                nc.gpsimd.collective_compute(
                    "AllGather",
                    mybir.AluOpType.bypass,
                    replica_groups=[[0, 1, 2, 3], [4, 5, 6, 7]],
                    ins=[att_loc[pair][h][:].opt()],
                    outs=[att_all[pair][h][:].opt()],
                )

        def emit_pv(ent):
            eAB_p, ik, nch, pair, jq, Wc = ent
            nc.tensor.matmul(
                qB[:, 0, Wc:512], V[:, ik, 0, :], eAB_p[:, 0, :],
                start=(ik == 0), stop=(ik == nch - 1),
            )
            nc.tensor.matmul(
                qB[:, 1, Wc:512], V[:, ik, 1, :], eAB_p[:, 1, :],
                start=(ik == 0), stop=(ik == nch - 1),
            )
            if ik + 1 >= nch:  # last chunk of this (pair, jq)
                emit_norm(pair, jq)

        def _cht_load(t, h):
            nc.gpsimd.dma_start(
                cht[t][h][:],
                att_all[t][h][:].rearrange("(r p) q -> p r q", p=P),
            )

        def emit_wo_loads():
            # wo weights + gathered-chunk preloads, spread across pair
            # 3's minis. Pair 3's own chunk loads ride the sync queue so
            # a still-flying AG never blocks the gpsimd stream that
            # carries the next AllGather triggers.
            nc.gpsimd.dma_start(
                wo_sb[:], wo.ap().rearrange("p (c k) -> p c k", c=DCH)
            )
            yield None
            for t in range(3):
                _cht_load(t, 0)
                yield None
            for t in range(2):
                _cht_load(t, 1)
                yield None
            for _ in range(8):
                yield None
            _cht_load(2, 1)
            yield None
            for _ in range(6):
                yield None
            nc.sync.dma_start(
                cht[3][0][:],
                att_all[3][0][:].rearrange("(r p) q -> p r q", p=P),
            )
            yield None

        pend = []  # global software pipeline: scores/exp run 1 ahead of PV
        filler = None
        for pair in range(NPAIR):
            filler = emit_qt(pair + 1) if pair + 1 < NPAIR else emit_wo_loads()
            for jq in range(NJQ):
                nch = 4 * jq + 4
                for ik in range(nch):
                    d = ik - 4 * jq
                    diag = d >= 0
                    Wc = 128 * d if diag else 0  # per-chunk causal window
                    # two alternating 2-bank score tiles: this chunk's
                    # scores only wait exp(t-2), so exp(t-1) overlaps
                    sAB = scp.tile([P, 2, 512], f32, tag=f"sa{ik % 2}",
                                   name="sAB", bufs=1)
                    if diag:
                        # seed the causal triangle with -1e5 (trib^T via
                        # identity); exp underflows it to exactly 0
                        for s_ in (0, 1):
                            nc.tensor.matmul(
                                sAB[:, s_, Wc : Wc + 128], trib_sb[:],
                                ident_sb[:], start=True, stop=False,
                            )
                    kap = KT[:, ik // 4, (ik % 4) * 128 : (ik % 4 + 1) * 128]
                    qap = QT[pair][:, jq, Wc:512]
                    nc.tensor.matmul(
                        sAB[:, 0, Wc:512], kap[0:64, :], qap[0:64, :],
                        start=not diag, stop=True,
                    )
                    nc.tensor.matmul(
                        sAB[:, 1, Wc:512], kap[64:128, :], qap[64:128, :],
                        start=not diag, stop=True,
                    )
                    eAB = ex.tile([P, 2, 512 - Wc], bf16, tag="eAB",
                                  name="eAB", bufs=5)
                    nc.scalar.activation(eAB[:], sAB[:, :, Wc:512], EXP,
                                         scale=0.125)
                    pend.append((eAB, ik, nch, pair, jq, Wc))
                    if len(pend) > 2:
                        emit_pv(pend.pop(0))
                    if filler is not None:
                        # PE filler; emitted last so its DVE ropes queue
                        # behind the norm copies that gate PV
                        if next(filler, StopIteration) is StopIteration:
                            filler = None
            if filler is not None:  # drain leftover projection work
                for _ in filler:
                    pass
                filler = None
            if pair == 2:
                # drain the PV pipeline so pair 2's last AllGather is
                # emitted before the chunk preloads that read it
                while pend:
                    emit_pv(pend.pop(0))
                # xT no longer needed; free it so the wo tiles can take
                # its address range
                proj_ctx.close()
                wos = ctx.enter_context(tc.tile_pool(name="wos", bufs=1))
                wo_sb = wos.tile([P, DCH, OCOLS], bf16)
                for t in range(NPAIR):
                    for h in range(2):
                        cht[t][h] = wos.tile(
                            [P, GRP, 1024], bf16, name=f"cht{t}_{h}",
                            tag=f"cht{t}_{h}",
                        )
        while pend:
            emit_pv(pend.pop(0))
        # gpsimd is free of critical work now (all AG triggers emitted)
        nc.gpsimd.dma_start(
            cht[3][1][:], att_all[3][1][:].rearrange("(r p) q -> p r q", p=P)
        )

        # ---- phase C: wo projection (lo/hi halves overlap last AGs) --
        # m-rows in interleaved pairs (consecutive matmuls alternate psum
        # banks); 4 independent psum tiles hold 4 m-pairs at once. The
        # contraction is split: t=0..2 chunks for ALL 4 pairs first (96
        # matmuls of runway that need no pair-3 data), then the t=3
        # tails + drains — so a still-flying pair-3 AllGather is hidden
        # behind the runway instead of stalling the whole phase.
        wops = [scp.tile([P, 2, 512], f32, tag=f"sa{g}", name=f"wop{g}", bufs=1)
                for g in range(2)] + [qA, qB]
        for h in range(2):
            for mp in range(4):     # partials: t = 0..2
                for c2 in range(12):
                    t, rr = c2 // 4, c2 % 4
                    for i in range(2):
                        mm = 2 * mp + i
                        nc.tensor.matmul(
                            wops[mp][:, i, :],
                            cht[t][h][:, rr, mm * P : (mm + 1) * P],
                            wo_sb[:, c2, :], start=(c2 == 0), stop=False,
                        )
            for mp in range(4):     # tails: t = 3, then drain
                for c2 in range(12, 16):
                    rr = c2 % 4
                    for i in range(2):
                        mm = 2 * mp + i
                        nc.tensor.matmul(
                            wops[mp][:, i, :],
                            cht[3][h][:, rr, mm * P : (mm + 1) * P],
                            wo_sb[:, c2, :], start=False, stop=(c2 == 15),
                        )
                for i in range(2):
                    m = h * 8 + 2 * mp + i
                    osb = wos.tile([P, OCOLS], f32, tag="osb", name="osb", bufs=4)
                    nc.vector.tensor_copy(out=osb[:], in_=wops[mp][:, i, :])
                    nc.sync.dma_start(
                        out.ap()[m * P : (m + 1) * P, :], osb[:]
                    )

        if dbg is not None:
            nc.sync.dma_start(dbg["kt_d"].ap(), KT[:].rearrange("p j q -> p (j q)"))
            nc.sync.dma_start(dbg["qt0_d"].ap(), QT[0][:].rearrange("p j q -> p (j q)"))
            nc.sync.dma_start(dbg["qt1_d"].ap(), QT[1][:].rearrange("p j q -> p (j q)"))
            nc.sync.dma_start(dbg["v_d"].ap(), V[:].rearrange("p c t v -> p (c t v)"))
            nc.sync.dma_start(dbg["att0_d"].ap(), attT[0][:])
            nc.sync.dma_start(dbg["att3_d"].ap(), attT[3][:])


# ---------------------------------------------------------------------------
# host side
# ---------------------------------------------------------------------------

_PERM64 = np.concatenate([np.arange(0, 64, 2), np.arange(1, 64, 2)])


def _qcols(r):
    cols = []
    for t in range(NREP):
        for half in range(2):
            h = (2 * r + half) * NREP + t
            cols.extend(64 * h + _PERM64)
    return np.array(cols)


def _kcols(r):
    cols = []
    for half in range(2):
        g = 2 * r + half
        cols.extend(64 * g + _PERM64)
    return np.array(cols)


def _worows():
    rows = []
    for t in range(NREP):
        for rr in range(GRP):
            for half in range(2):
                h = (2 * rr + half) * NREP + t
                rows.extend(64 * h + np.arange(64))
    return np.array(rows)


def _panel(w):
    """[DCH*P, N] -> [P, DCH*N] (chunk-major per partition)."""
    n = w.shape[1]
    return np.ascontiguousarray(
        w.reshape(DCH, P, n).transpose(1, 0, 2).reshape(P, DCH * n)
    )


def make_in_maps(x, wq, wk, wv, wo, freqs_cos, freqs_sin):
    cosT = np.ascontiguousarray(freqs_cos.T).astype(np.float32)  # (32, S)
    sinT = np.ascontiguousarray(freqs_sin.T).astype(np.float32)
    cos4 = np.ascontiguousarray(np.tile(cosT, (4, 1))).astype(_BF)  # (128, S)
    # pre-swapped by 32-blocks: partition p holds the sin coefficient
    # for the rope output at partition swap32(p)
    sin4 = np.ascontiguousarray(
        np.concatenate([sinT, -sinT, sinT, -sinT], axis=0)
    ).astype(_BF)
    trib = (-1e5 * np.triu(np.ones((P, P), dtype=np.float32), 1)).astype(_BF)
    ident = np.eye(P, dtype=np.float32).astype(_BF)

    xT = [np.ascontiguousarray(x[b].T).astype(_BF) for b in range(B)]
    wo_perm = wo[_worows(), :]

    in_maps = []
    for c in range(NCORES):
        b, r = c // GRP, c % GRP
        wq_r = wq[:, _qcols(r)].astype(_BF)      # [D, 512], tile-major cols
        wq_pan = np.concatenate(
            [_panel(wq_r[:, t * P : (t + 1) * P]) for t in range(NPAIR)], axis=1
        )
        in_maps.append(
            {
                "xT": xT[b],
                "wq": np.ascontiguousarray(wq_pan),
                "wk": _panel(wk[:, _kcols(r)].astype(_BF)),
                "wv": _panel(wv[:, 128 * r : 128 * (r + 1)].astype(_BF)),
                "wo": _panel(
                    wo_perm[:, OCOLS * r : OCOLS * (r + 1)].astype(_BF)
                ),
                "cos4": cos4,
                "sin4": sin4,
                "trib": trib,
                "ident": ident,
            }
        )
    return in_maps


_NC_CACHE = None


def _get_nc():
    global _NC_CACHE
    if _NC_CACHE is None:
        _NC_CACHE = build_graph()
    return _NC_CACHE


def kernel(x, wq, wk, wv, wo, freqs_cos, freqs_sin):
    x = np.asarray(x)
    wq = np.asarray(wq)
    wk = np.asarray(wk)
    wv = np.asarray(wv)
    wo = np.asarray(wo)
    freqs_cos = np.asarray(freqs_cos)
    freqs_sin = np.asarray(freqs_sin)

    in_maps = make_in_maps(x, wq, wk, wv, wo, freqs_cos, freqs_sin)
    nc = _get_nc()
    res = run_bass_kernel_spmd(nc, in_maps, core_ids=list(range(NCORES)))
    global _LAST_RES
    _LAST_RES = res

    out = np.empty((B, S, D), dtype=np.float32)
    for c in range(NCORES):
        b, r = c // GRP, c % GRP
        out[b, :, OCOLS * r : OCOLS * (r + 1)] = res.results[c]["out"]
    return out

